# revision 1
# baseline (speedup 1.0000x reference)
"""Trainium2 Bass kernel for MHSA with Transformer-XL relative position bias.

Problem: B=16, T=1024, DM=256, H=4, HS=64 fp32.
Sharding: pure data-parallel over batch across 8 cores (2 batches/core).

Per-core pipeline (M = 2*1024 = 2048 rows):
  1. LN in [m, d] layout (bn_stats), PE-transpose -> xnT/posT [256, M] bf16
  2. Projections via PE: QuT/QvT/KT/PT [256, M] bf16 (s on partitions), V [M, 256] bf16
  3. Per (b, h): pos scores X = QvT.T @ PT -> PSUM -> bf16 -> DRAM scratch
     [1024, 1025] (col 0 zeroed); rel_shift = re-read with row-stride 1024 from
     element offset 1024 (Transformer-XL shift == flat-buffer shear);
     content scores C = QuT.T @ KT into PSUM, R added into the same PSUM via
     identity matmul; exp((C+R)/8) on ACT with fused row-sum (logits are small:
     max |logit| ~ 1.2, so no max subtraction); A = E * (1/S) in bf16;
     A transposed n<->m via xbar DMA (SBUF->SBUF); AV^T accumulated on PE.
  4. Out-proj from AVT (f32r), + bo + residual, DMA out.
"""
import sys

sys.path.insert(0, "/opt/trn_rl_repo")

import numpy as np

import concourse.bass as bass
import concourse.bacc as bacc
import concourse.tile as tile
from concourse import mybir
from concourse.masks import make_identity
from concourse.bass_utils import run_bass_kernel_spmd

B, T, DM, H, HS = 16, 1024, 256, 4, 64
NCORES = 8
BL = B // NCORES          # local batches per core
M = BL * T                # local rows (2048)
NMT = M // 128            # m-tiles (16)
P = 128
LN_EPS = 1e-3
F32 = mybir.dt.float32
F32R = mybir.dt.float32r
BF16 = mybir.dt.bfloat16


def build_bass():
    nc = bacc.Bacc("TRN2", target_bir_lowering=False, debug=False,
                   enable_asserts=False, num_devices=NCORES)

    x_in = nc.dram_tensor("x", [M, DM], F32, kind="ExternalInput").ap()
    pos_in = nc.dram_tensor("pos", [M, DM], F32, kind="ExternalInput").ap()
    wq_in = nc.dram_tensor("wq", [DM, DM], F32, kind="ExternalInput").ap()
    wk_in = nc.dram_tensor("wk", [DM, DM], F32, kind="ExternalInput").ap()
    wv_in = nc.dram_tensor("wv", [DM, DM], F32, kind="ExternalInput").ap()
    wp_in = nc.dram_tensor("wp", [DM, DM], F32, kind="ExternalInput").ap()
    wo_in = nc.dram_tensor("wo", [DM, DM], F32, kind="ExternalInput").ap()
    bqu_in = nc.dram_tensor("bqu", [DM], F32, kind="ExternalInput").ap()
    bqv_in = nc.dram_tensor("bqv", [DM], F32, kind="ExternalInput").ap()
    bk_in = nc.dram_tensor("bk", [DM], F32, kind="ExternalInput").ap()
    bo_in = nc.dram_tensor("bo", [DM], F32, kind="ExternalInput").ap()
    out = nc.dram_tensor("out", [M, DM], F32, kind="ExternalOutput").ap()

    scr = [
        nc.dram_tensor(f"xscr{i}", [T, T + 1], BF16, kind="Internal").ap()
        for i in range(2)
    ]

    with tile.TileContext(nc) as tc:
        with tc.tile_pool(name="persist", bufs=1) as pp:
            # --- persistent SBUF ---
            ident = pp.tile([P, P], F32)
            make_identity(nc, ident)
            ident_bf = pp.tile([P, P], BF16)
            nc.gpsimd.tensor_copy(out=ident_bf, in_=ident)

            def load_w(ap_in, dtype, name):
                ts = [pp.tile([P, DM], dtype, tag=f"{name}{c}", name=f"{name}{c}") for c in range(2)]
                for c in range(2):
                    if dtype == F32:
                        nc.sync.dma_start(out=ts[c], in_=ap_in[c * P:(c + 1) * P, :])
                    else:
                        tmp = pp.tile([P, DM], F32, tag=f"{name}tmp{c}", name=f"{name}tmp{c}")
                        nc.sync.dma_start(out=tmp, in_=ap_in[c * P:(c + 1) * P, :])
                        nc.gpsimd.tensor_copy(out=ts[c], in_=tmp)
                return ts

            wq_sb = load_w(wq_in, BF16, "wq")
            wk_sb = load_w(wk_in, BF16, "wk")
            wv_sb = load_w(wv_in, BF16, "wv")
            wp_sb = load_w(wp_in, BF16, "wp")
            wo_sb = load_w(wo_in, BF16, "wo")

            def load_col(ap_in, name):
                ts = [pp.tile([P, 1], F32, tag=f"{name}{c}", name=f"{name}{c}") for c in range(2)]
                for c in range(2):
                    nc.sync.dma_start(
                        out=ts[c],
                        in_=bass.AP(tensor=ap_in.tensor, offset=c * P, ap=[[1, P], [1, 1]]),
                    )
                return ts

            bqu_c = load_col(bqu_in, "bqu")
            bqv_c = load_col(bqv_in, "bqv")
            bk_c = load_col(bk_in, "bk")

            def load_bcast(ap_in, name):
                t = pp.tile([P, DM], F32, tag=f"{name}b", name=f"{name}b")
                nc.sync.dma_start(
                    out=t,
                    in_=bass.AP(tensor=ap_in.tensor, offset=0, ap=[[0, P], [1, DM]]),
                )
                return t

            bo_b = load_bcast(bo_in, "bo")

            eps_t = pp.tile([P, 1], F32)
            nc.vector.memset(eps_t, LN_EPS)

            x_res = pp.tile([P, NMT, DM], F32)        # residual copy of inputs
            xnT = [pp.tile([P, M], BF16, tag=f"xnT{c}", name=f"xnT{c}") for c in range(2)]
            posT = [pp.tile([P, M], BF16, tag=f"posT{c}", name=f"posT{c}") for c in range(2)]
            quT = [pp.tile([P, M], BF16, tag=f"quT{c}", name=f"quT{c}") for c in range(2)]
            qvT = [pp.tile([P, M], BF16, tag=f"qvT{c}", name=f"qvT{c}") for c in range(2)]
            kT = [pp.tile([P, M], BF16, tag=f"kT{c}", name=f"kT{c}") for c in range(2)]
            pT = [pp.tile([P, M], BF16, tag=f"pT{c}", name=f"pT{c}") for c in range(2)]
            v_sb = pp.tile([P, NMT, DM], BF16)        # V[mt*128+p, s] at [:, mt, s]
            avT = [pp.tile([P, M], BF16, tag=f"avT{c}", name=f"avT{c}") for c in range(2)]

            # ---------------- phase 1: LN + transposes ----------------
            with tc.tile_pool(name="ph1", bufs=3) as sb1, \
                 tc.tile_pool(name="ps1", bufs=4, space="PSUM") as ps1:
                for mt in range(NMT):
                    xs = x_res[:, mt, :]
                    nc.sync.dma_start(out=xs, in_=x_in[mt * P:(mt + 1) * P, :])
                    stats = sb1.tile([P, 6], F32, tag="stats")
                    nc.vector.bn_stats(out=stats, in_=xs)
                    mv = sb1.tile([P, 2], F32, tag="mv")
                    nc.vector.bn_aggr(out=mv, in_=stats)
                    rstd = sb1.tile([P, 1], F32, tag="rstd")
                    nc.scalar.activation(out=rstd, in_=mv[:, 1:2],
                                         func=mybir.ActivationFunctionType.Sqrt,
                                         bias=eps_t, scale=1.0)
                    nc.vector.reciprocal(out=rstd, in_=rstd)
                    xn = sb1.tile([P, DM], F32, tag="xn")
                    nc.vector.tensor_scalar(out=xn, in0=xs, scalar1=mv[:, 0:1],
                                            scalar2=rstd,
                                            op0=mybir.AluOpType.subtract,
                                            op1=mybir.AluOpType.mult)
                    pt = sb1.tile([P, DM], F32, tag="pt")
                    nc.sync.dma_start(out=pt, in_=pos_in[mt * P:(mt + 1) * P, :])
                    for c in range(2):
                        tp = ps1.tile([P, P], F32, tag="tp")
                        nc.tensor.transpose(tp, xn[:, c * P:(c + 1) * P], ident)
                        nc.scalar.copy(out=xnT[c][:, mt * P:(mt + 1) * P], in_=tp)
                        tp2 = ps1.tile([P, P], F32, tag="tp")
                        nc.tensor.transpose(tp2, pt[:, c * P:(c + 1) * P], ident)
                        nc.scalar.copy(out=posT[c][:, mt * P:(mt + 1) * P], in_=tp2)

            # ---------------- phase 2: projections ----------------
            with tc.tile_pool(name="ps2", bufs=2, space="PSUM") as ps2:
                for sc in range(2):
                    for mc in range(4):
                        msl = slice(mc * 512, (mc + 1) * 512)
                        pq = ps2.tile([P, 512], F32, tag="pq")
                        pk = ps2.tile([P, 512], F32, tag="pk")
                        pps = ps2.tile([P, 512], F32, tag="pp")
                        for dc in range(2):
                            nc.tensor.matmul(pq, lhsT=wq_sb[dc][:, sc * P:(sc + 1) * P],
                                             rhs=xnT[dc][:, msl],
                                             start=(dc == 0), stop=(dc == 1))
                            nc.tensor.matmul(pk, lhsT=wk_sb[dc][:, sc * P:(sc + 1) * P],
                                             rhs=xnT[dc][:, msl],
                                             start=(dc == 0), stop=(dc == 1))
                            nc.tensor.matmul(pps, lhsT=wp_sb[dc][:, sc * P:(sc + 1) * P],
                                             rhs=posT[dc][:, msl],
                                             start=(dc == 0), stop=(dc == 1))
                        nc.scalar.activation(out=quT[sc][:, msl], in_=pq,
                                             func=mybir.ActivationFunctionType.Identity,
                                             bias=bqu_c[sc], scale=1.0)
                        nc.scalar.activation(out=qvT[sc][:, msl], in_=pq,
                                             func=mybir.ActivationFunctionType.Identity,
                                             bias=bqv_c[sc], scale=1.0)
                        nc.vector.tensor_scalar_add(out=kT[sc][:, msl], in0=pk,
                                                    scalar1=bk_c[sc])
                        nc.vector.tensor_copy(out=pT[sc][:, msl], in_=pps)
                for mt in range(NMT):
                    pv = ps2.tile([P, DM], F32, tag="pv")
                    for dc in range(2):
                        nc.tensor.matmul(pv, lhsT=xnT[dc][:, mt * P:(mt + 1) * P],
                                         rhs=wv_sb[dc],
                                         start=(dc == 0), stop=(dc == 1))
                    nc.vector.tensor_copy(out=v_sb[:, mt, :], in_=pv)

            # ---------------- phase 3: attention per (b, h) ----------------
            with tc.tile_pool(name="ph3", bufs=3) as sb3, \
                 tc.tile_pool(name="at", bufs=2) as atp, \
                 tc.tile_pool(name="ps3", bufs=2, space="PSUM") as ps3, \
                 tc.tile_pool(name="ps3x", bufs=2, space="PSUM") as ps3x, \
                 tc.tile_pool(name="ps3av", bufs=2, space="PSUM") as ps3av:
                NBH = BL * H
                at_tiles = {}

                def stage_a(bh, mt):
                    b, h = divmod(bh, H)
                    hh, po = h // 2, (h % 2) * 64
                    ssl = slice(po, po + 64)
                    sc_t = scr[bh % 2]
                    mg = slice(b * T + mt * P, b * T + (mt + 1) * P)
                    xbf = sb3.tile([P, T + 1], BF16, tag="xbf", name="xbf")
                    nc.gpsimd.memset(xbf[:, 0:1], 0.0)
                    for nck in range(2):
                        xp = ps3x.tile([P, 512], F32, tag="xp", name="xp")
                        nc.tensor.matmul(
                            xp, lhsT=qvT[hh][ssl, mg],
                            rhs=pT[hh][ssl, b * T + nck * 512:b * T + (nck + 1) * 512],
                            start=True, stop=True)
                        osl = xbf[:, 1 + nck * 512:1 + (nck + 1) * 512]
                        if nck == 0:
                            nc.vector.tensor_copy(out=osl, in_=xp)
                        else:
                            nc.scalar.copy(out=osl, in_=xp)
                    nc.gpsimd.dma_start(out=sc_t[mt * P:(mt + 1) * P, :], in_=xbf)

                def stage_bc(bh, mt):
                    b, h = divmod(bh, H)
                    hh, po = h // 2, (h % 2) * 64
                    ssl = slice(po, po + 64)
                    sc_t = scr[bh % 2]
                    at = at_tiles[bh]
                    mg = slice(b * T + mt * P, b * T + (mt + 1) * P)
                    rbf = sb3.tile([P, T], BF16, tag="rbf", name="rbf")
                    nc.sync.dma_start(
                        out=rbf,
                        in_=bass.AP(tensor=sc_t.tensor, offset=T + mt * P * T,
                                    ap=[[T, P], [1, T]]))
                    cp = ps3.tile([P, T], F32, tag="big", name="cp")
                    for nck in range(2):
                        nc.tensor.matmul(
                            cp[:, nck * 512:(nck + 1) * 512], lhsT=quT[hh][ssl, mg],
                            rhs=kT[hh][ssl, b * T + nck * 512:b * T + (nck + 1) * 512],
                            start=True, stop=True)
                    lbf = sb3.tile([P, T], BF16, tag="lbf", name="lbf")
                    nc.vector.scalar_tensor_tensor(
                        out=lbf, in0=cp, scalar=0.0, in1=rbf,
                        op0=mybir.AluOpType.bypass, op1=mybir.AluOpType.add)
                    ebf = sb3.tile([P, T], BF16, tag="ebf", name="ebf")
                    ssum = sb3.tile([P, 1], F32, tag="ssum", name="ssum")
                    nc.scalar.activation(out=ebf, in_=lbf,
                                         func=mybir.ActivationFunctionType.Exp,
                                         scale=0.125, accum_out=ssum)
                    nc.vector.reciprocal(out=ssum, in_=ssum)
                    abf = sb3.tile([P, T], BF16, tag="abf", name="abf")
                    nc.vector.tensor_scalar_mul(out=abf, in0=ebf, scalar1=ssum)
                    nc.sync.dma_start_transpose(
                        out=at[:, :, mt * P:(mt + 1) * P], in_=abf)

                def stage_d(bh, avps, nt):
                    b, h = divmod(bh, H)
                    at = at_tiles[bh]
                    for mc in range(2):
                        nc.tensor.matmul(
                            avps[mc],
                            lhsT=v_sb[:, b * (T // P) + nt, h * HS:(h + 1) * HS],
                            rhs=at[:, nt, mc * 512:(mc + 1) * 512],
                            start=(nt == 0), stop=(nt == T // P - 1))

                def stage_d_out(bh, avps):
                    b, h = divmod(bh, H)
                    hh, po = h // 2, (h % 2) * 64
                    for mc in range(2):
                        nc.scalar.copy(
                            out=avT[hh][po:po + 64,
                                        b * T + mc * 512:b * T + (mc + 1) * 512],
                            in_=avps[mc])
                    del at_tiles[bh]

                avps_cur = None
                for step in range(NBH + 2):
                    if step - 1 >= 0 and step - 1 < NBH:
                        at_tiles[step - 1] = atp.tile([P, T // P, T], BF16,
                                                      tag="at", name="at")
                    if step - 2 >= 0:
                        avps_cur = [ps3av.tile([64, 512], F32, tag="av",
                                               name=f"avp{mc}") for mc in range(2)]
                    for mt in range(T // P):
                        if step < NBH:
                            stage_a(step, mt)
                        if 0 <= step - 1 < NBH:
                            stage_bc(step - 1, mt)
                        if step - 2 >= 0:
                            stage_d(step - 2, avps_cur, mt)
                    if step - 2 >= 0:
                        stage_d_out(step - 2, avps_cur)

            # ---------------- phase 4: out-proj + residual ----------------
            with tc.tile_pool(name="ph4", bufs=3) as sb4, \
                 tc.tile_pool(name="ps4", bufs=2, space="PSUM") as ps4:
                for mt in range(NMT):
                    op = ps4.tile([P, DM], F32, tag="op")
                    for sc in range(2):
                        nc.tensor.matmul(op,
                                         lhsT=avT[sc][:, mt * P:(mt + 1) * P],
                                         rhs=wo_sb[sc],
                                         start=(sc == 0), stop=(sc == 1))
                    ot = sb4.tile([P, DM], F32, tag="ot")
                    nc.vector.scalar_tensor_tensor(out=ot, in0=op, scalar=0.0,
                                                   in1=x_res[:, mt, :],
                                                   op0=mybir.AluOpType.bypass,
                                                   op1=mybir.AluOpType.add)
                    nc.vector.tensor_tensor(out=ot, in0=ot, in1=bo_b,
                                            op=mybir.AluOpType.add)
                    nc.sync.dma_start(out=out[mt * P:(mt + 1) * P, :], in_=ot)
    nc.finalize()
    return nc


_NC = None


def make_in_maps(inputs):
    f = lambda a: np.ascontiguousarray(np.asarray(a, dtype=np.float32))
    x = f(inputs["inputs"]).reshape(B, T, DM)
    pos = f(inputs["pos_enc"]).reshape(B, T, DM)
    wq0 = f(inputs["Wq"]).reshape(DM, DM)
    wk0 = f(inputs["Wk"]).reshape(DM, DM)
    wv0 = f(inputs["Wv"]).reshape(DM, DM)
    wp = f(inputs["Wp"]).reshape(DM, DM)
    wo = f(inputs["Wo"]).reshape(DM, DM)
    gamma = f(inputs["gamma"]).reshape(DM, 1)
    beta = f(inputs["beta"]).reshape(DM)
    # fold LN's gamma into the x-side weights, beta into the projection biases,
    # and bv through softmax (rows sum to 1) into the output bias
    wq, wk, wv = gamma * wq0, gamma * wk0, gamma * wv0
    bqu = (f(inputs["bq"]).reshape(DM) + f(inputs["pos_bias_u"]).reshape(DM)
           + beta @ wq0)
    bqv = (f(inputs["bq"]).reshape(DM) + f(inputs["pos_bias_v"]).reshape(DM)
           + beta @ wq0)
    bk = f(inputs["bk"]).reshape(DM) + beta @ wk0
    bv_eff = f(inputs["bv"]).reshape(DM) + beta @ wv0
    bo = f(inputs["bo"]) + bv_eff @ wo
    shared = dict(
        wq=wq, wk=wk, wv=wv, wp=wp, wo=wo,
        bqu=bqu, bqv=bqv, bk=bk, bo=bo,
    )
    in_maps = []
    for c in range(NCORES):
        sl = slice(c * BL, (c + 1) * BL)
        in_maps.append(dict(
            x=np.ascontiguousarray(x[sl].reshape(M, DM)),
            pos=np.ascontiguousarray(pos[sl].reshape(M, DM)),
            **shared,
        ))
    return in_maps


def kernel(**inputs) -> np.ndarray:
    global _NC
    if _NC is None:
        _NC = build_bass()
    in_maps = make_in_maps(inputs)
    res = run_bass_kernel_spmd(_NC, in_maps, core_ids=list(range(NCORES)))
    outs = [r["out"].reshape(BL, T, DM) for r in res.results]
    return np.concatenate(outs, axis=0)



# revision 10
# speedup vs baseline: 1.0344x; 1.0344x over previous
"""Trainium2 Bass kernel for MHSA with Transformer-XL relative position bias.

Problem: B=16, T=1024, DM=256, H=4, HS=64 fp32.
Sharding: pure data-parallel over batch across 8 cores (2 batches/core).

v2 design (fp8 DoubleRow everywhere upstream):
  - LN in f32 -> xn bf16 -> PE transpose bf16 -> xnT8/posT8 fp8 [128, 2dc, M]
    (d packed as 2 k-subtiles for DoubleRow).
  - Projections: one fp8-DR matmul per psum; W columns host-permuted so the
    q/k head-fold layout [32h+k partitions, j subtile] falls out of the psum.
    Weights scaled x16 on host (fp8 subnormal avoidance), 1/16 folded into
    the psum->fp8 copies.
  - Scores per (b,h): X = qv.P^T and C = qu.K^T via fp8-DR [K=32x2] matmuls;
    X -> bf16 -> DRAM scratch [1024,1025] (col0 = 0); rel_shift = strided
    re-read (flat-buffer shear). R added into C-psum via bf16 identity
    matmul; ACT exp from psum -> E fp8 + exact f32 row-sum accum.
  - E^T via xbar DMA transpose of E viewed as u16 pairs -> at [128,4c,1024m]
    (logical row r = c*128+p => n = 256c+2p+j). V stored pre-packed to the
    matching layout; AV = fp8-DR over (c, j) -> avps [64, 1024] psum.
  - Softmax normalization deferred: row-sums -> recip -> PE-transpose ->
    fold -> partition_broadcast -> recB [128, m]; avT = avps * recB (bf16).
  - Out-proj bf16 from avT + residual + bo.
"""
import sys

sys.path.insert(0, "/opt/trn_rl_repo")

import numpy as np

import concourse.bass as bass
import concourse.bacc as bacc
import concourse.tile as tile
from concourse import mybir
from concourse.masks import make_identity
from concourse.bass_utils import run_bass_kernel_spmd

B, T, DM, H, HS = 16, 1024, 256, 4, 64
NCORES = 8
BL = B // NCORES          # local batches per core (2)
M = BL * T                # local rows (2048)
NMT = M // 128            # m-tiles (16)
P = 128
NBH = BL * H              # 8 (b,h) pairs per core
NSCR = 4                  # rotating DRAM scratch buffers
LN_EPS = 1e-3
WSCALE = 16.0             # host-side weight scale (fp8 subnormal avoidance)
F32 = mybir.dt.float32
BF16 = mybir.dt.bfloat16
FP8 = mybir.dt.float8e4
U16 = mybir.dt.uint16
DR = mybir.MatmulPerfMode.DoubleRow
EXP = mybir.ActivationFunctionType.Exp
IDENT = mybir.ActivationFunctionType.Identity
SQRT = mybir.ActivationFunctionType.Sqrt
ADD = mybir.AluOpType.add
MULT = mybir.AluOpType.mult
SUB = mybir.AluOpType.subtract
BYPASS = mybir.AluOpType.bypass


def build_bass():
    nc = bacc.Bacc("TRN2", target_bir_lowering=False, debug=False,
                   enable_asserts=False, num_devices=NCORES)

    x_in = nc.dram_tensor("x", [M, DM], F32, kind="ExternalInput").ap()
    pos_in = nc.dram_tensor("pos", [M, DM], F32, kind="ExternalInput").ap()
    wq_in = nc.dram_tensor("wq", [2, P, DM], F32, kind="ExternalInput").ap()
    wk_in = nc.dram_tensor("wk", [2, P, DM], F32, kind="ExternalInput").ap()
    wp_in = nc.dram_tensor("wp", [2, P, DM], F32, kind="ExternalInput").ap()
    wv_in = nc.dram_tensor("wv", [2, P, DM], F32, kind="ExternalInput").ap()
    wo_in = nc.dram_tensor("wo", [4, 64, DM], F32, kind="ExternalInput").ap()
    bvec_in = nc.dram_tensor("bvec", [6, P], F32, kind="ExternalInput").ap()
    bo_in = nc.dram_tensor("bo", [DM], F32, kind="ExternalInput").ap()
    out = nc.dram_tensor("out", [M, DM], F32, kind="ExternalOutput").ap()

    scr = [
        nc.dram_tensor(f"xscr{i}", [T, T + 1], BF16, kind="Internal").ap()
        for i in range(NSCR)
    ]

    with tile.TileContext(nc) as tc:
        with tc.tile_pool(name="persist", bufs=1) as pp:
            # --- persistent SBUF ---
            ident = pp.tile([P, P], F32)
            make_identity(nc, ident)
            id_bf = pp.tile([P, P], BF16)
            nc.gpsimd.tensor_copy(out=id_bf, in_=ident)

            def load_w8(ap_in, name):
                tmp = pp.tile([P, 2, DM], F32, tag=f"{name}t", name=f"{name}t")
                for dc in range(2):
                    nc.sync.dma_start(out=tmp[:, dc, :], in_=ap_in[dc])
                w8 = pp.tile([P, 2, DM], FP8, tag=name, name=name)
                nc.gpsimd.tensor_copy(out=w8, in_=tmp)
                return w8

            w8q = load_w8(wq_in, "w8q")
            w8k = load_w8(wk_in, "w8k")
            w8p = load_w8(wp_in, "w8p")
            w8v = load_w8(wv_in, "w8v")

            wo_sb = []
            for h_ in range(4):
                tmp = pp.tile([64, DM], F32, tag=f"wot{h_}", name=f"wot{h_}")
                nc.sync.dma_start(out=tmp, in_=wo_in[h_])
                t = pp.tile([64, DM], BF16, tag=f"wo{h_}", name=f"wo{h_}")
                nc.gpsimd.tensor_copy(out=t, in_=tmp)
                wo_sb.append(t)

            def load_col(row, name):
                t = pp.tile([P, 1], F32, tag=name, name=name)
                nc.sync.dma_start(
                    out=t,
                    in_=bass.AP(tensor=bvec_in.tensor, offset=row * P,
                                ap=[[1, P], [1, 1]]),
                )
                return t

            bquA = load_col(0, "bquA")
            bquB = load_col(1, "bquB")
            dqvA = load_col(2, "dqvA")
            dqvB = load_col(3, "dqvB")
            bkA = load_col(4, "bkA")
            bkB = load_col(5, "bkB")

            bo_b = pp.tile([P, DM], F32, tag="bo_b", name="bo_b")
            nc.sync.dma_start(
                out=bo_b,
                in_=bass.AP(tensor=bo_in.tensor, offset=0, ap=[[0, P], [1, DM]]),
            )

            eps_t = pp.tile([P, 1], F32)
            nc.vector.memset(eps_t, LN_EPS)

            x_res = pp.tile([P, NMT, DM], F32)
            xnT8 = pp.tile([P, 2, M], FP8)
            posT8 = pp.tile([P, 2, M], FP8)
            qu8 = pp.tile([P, 2, M], FP8)
            qv8 = pp.tile([P, 2, M], FP8)
            k8 = pp.tile([P, 2, M], FP8)
            p8 = pp.tile([P, 2, M], FP8)
            # V packed for AV DoubleRow: v8p[p, b, c, j, s] = V[b, 256c+2p+j, s]
            v8p = pp.tile([P, BL, 4, 2, DM], FP8)
            avT = [pp.tile([64, M], BF16, tag=f"avT{c}", name=f"avT{c}")
                   for c in range(4)]

            xbf_ring = [pp.tile([P, T + 1], BF16, tag=f"xbf{i}", name=f"xbf{i}")
                        for i in range(3)]
            for t in xbf_ring:
                nc.gpsimd.memset(t[:, 0:1], 0.0)

            # ---------------- phase 1+2: LN, transposes, projections --------
            with tc.tile_pool(name="ph1", bufs=3) as sb1, \
                 tc.tile_pool(name="ps1", bufs=2, space="PSUM") as ps1, \
                 tc.tile_pool(name="ps2", bufs=2, space="PSUM") as ps2, \
                 tc.tile_pool(name="psv", bufs=2, space="PSUM") as psv:

                def ph1_body(mt):
                    xs = x_res[:, mt, :]
                    nc.sync.dma_start(out=xs, in_=x_in[mt * P:(mt + 1) * P, :])
                    stats = sb1.tile([P, 6], F32, tag="stats")
                    nc.vector.bn_stats(out=stats, in_=xs)
                    mv = sb1.tile([P, 2], F32, tag="mv")
                    nc.vector.bn_aggr(out=mv, in_=stats)
                    rstd = sb1.tile([P, 1], F32, tag="rstd")
                    nc.scalar.activation(out=rstd, in_=mv[:, 1:2], func=SQRT,
                                         bias=eps_t, scale=1.0)
                    nc.vector.reciprocal(out=rstd, in_=rstd)
                    xnb = sb1.tile([P, DM], BF16, tag="xnb")
                    nc.vector.tensor_scalar(out=xnb, in0=xs,
                                            scalar1=mv[:, 0:1], scalar2=rstd,
                                            op0=SUB, op1=MULT)
                    pt = sb1.tile([P, DM], F32, tag="pt")
                    nc.sync.dma_start(out=pt, in_=pos_in[mt * P:(mt + 1) * P, :])
                    pb = sb1.tile([P, DM], BF16, tag="pb")
                    nc.gpsimd.tensor_copy(out=pb, in_=pt)
                    msl = slice(mt * P, (mt + 1) * P)
                    for c in range(2):
                        tp = ps1.tile([P, P], BF16, tag="tp")
                        nc.tensor.transpose(tp, xnb[:, c * P:(c + 1) * P], id_bf)
                        nc.scalar.copy(out=xnT8[:, c, msl], in_=tp)
                        tq = ps1.tile([P, P], BF16, tag="tp", name="tq")
                        nc.tensor.transpose(tq, pb[:, c * P:(c + 1) * P], id_bf)
                        nc.vector.tensor_copy(out=posT8[:, c, msl], in_=tq)

                def ph2_chunk(mc):
                    msl = slice(mc * 512, (mc + 1) * 512)
                    ivw = 1.0 / WSCALE

                    def proj(w8, rhsT, tag):
                        pa = ps2.tile([P, 512], F32, tag="prA", name="prA")
                        nc.tensor.matmul(pa, lhsT=w8[:, :, 0:P],
                                         rhs=rhsT[:, :, msl],
                                         start=True, stop=True, perf_mode=DR)
                        pb_ = ps2.tile([P, 512], F32, tag="prB", name="prB")
                        nc.tensor.matmul(pb_, lhsT=w8[:, :, P:2 * P],
                                         rhs=rhsT[:, :, msl],
                                         start=True, stop=True, perf_mode=DR)
                        return pa, pb_

                    qA, qB = proj(w8q, xnT8, "q")
                    kA, kB = proj(w8k, xnT8, "k")
                    pA, pB = proj(w8p, posT8, "p")
                    nc.scalar.activation(out=qu8[:, 0, msl], in_=qA, func=IDENT,
                                         bias=bquA, scale=ivw)
                    nc.scalar.activation(out=qu8[:, 1, msl], in_=qB, func=IDENT,
                                         bias=bquB, scale=ivw)
                    nc.scalar.mul(p8[:, 0, msl], pA, ivw)
                    nc.scalar.mul(p8[:, 1, msl], pB, ivw)
                    nc.vector.tensor_scalar(out=k8[:, 0, msl], in0=kA,
                                            scalar1=ivw, scalar2=bkA,
                                            op0=MULT, op1=ADD)
                    nc.vector.tensor_scalar(out=k8[:, 1, msl], in0=kB,
                                            scalar1=ivw, scalar2=bkB,
                                            op0=MULT, op1=ADD)
                    nc.vector.tensor_scalar_add(out=qv8[:, 0, msl],
                                                in0=qu8[:, 0, msl],
                                                scalar1=dqvA)
                    nc.vector.tensor_scalar_add(out=qv8[:, 1, msl],
                                                in0=qu8[:, 1, msl],
                                                scalar1=dqvB)
                    for mt in range(mc * 4, mc * 4 + 4):
                        pv = psv.tile([P, DM], F32, tag="pv")
                        nc.tensor.matmul(
                            pv, lhsT=xnT8[:, :, mt * P:(mt + 1) * P],
                            rhs=w8v, start=True, stop=True, perf_mode=DR)
                        v8s = sb1.tile([P, DM], FP8, tag="v8s")
                        if mt % 2 == 0:
                            nc.scalar.mul(v8s, pv, ivw)
                        else:
                            nc.vector.tensor_scalar_mul(out=v8s, in0=pv,
                                                        scalar1=ivw)
                        nc.gpsimd.dma_start(
                            out=v8p[64 * (mt % 2):64 * (mt % 2) + 64,
                                    mt // 8, (mt % 8) // 2, :, :],
                            in_=v8s)

                for mc in range(4):
                    for mt in range(mc * 4, mc * 4 + 4):
                        ph1_body(mt)
                    ph2_chunk(mc)

            # ---------------- phase 3: attention per (b, h) ------------------
            with tc.tile_pool(name="sb3", bufs=4) as sb3, \
                 tc.tile_pool(name="e8p", bufs=3) as e8p, \
                 tc.tile_pool(name="atp", bufs=2) as atp, \
                 tc.tile_pool(name="recp", bufs=2) as recp, \
                 tc.tile_pool(name="psx", bufs=2, space="PSUM") as psx, \
                 tc.tile_pool(name="psc", bufs=2, space="PSUM") as psc, \
                 tc.tile_pool(name="psav", bufs=1, space="PSUM") as psav, \
                 tc.tile_pool(name="ps4", bufs=1, space="PSUM") as ps4:

                at_tiles = {}
                rec_tiles = {}
                rbf_tiles = {}

                def stage_a(bh, mt):
                    b, h = divmod(bh, H)
                    hsl = slice(32 * h, 32 * h + 32)
                    mg = slice(b * T + mt * P, b * T + (mt + 1) * P)
                    xbf = xbf_ring[(bh * 8 + mt) % 3]
                    for nck in range(2):
                        xp = psx.tile([P, 512], F32, tag="x")
                        nc.tensor.matmul(
                            xp, lhsT=qv8[hsl, :, mg],
                            rhs=p8[hsl, :, b * T + nck * 512:b * T + (nck + 1) * 512],
                            start=True, stop=True, perf_mode=DR,
                            tile_position=(32 * h, 0))
                        nc.vector.tensor_copy(
                            out=xbf[:, 1 + nck * 512:1 + (nck + 1) * 512], in_=xp)
                    nc.sync.dma_start(out=scr[bh % NSCR][mt * P:(mt + 1) * P, :],
                                      in_=xbf)

                def issue_shears(bh):
                    sc_t = scr[bh % NSCR]
                    for mt in range(8):
                        t = sb3.tile([P, T], BF16, tag="rbf", name="rbf")
                        nc.gpsimd.dma_start(
                            out=t,
                            in_=bass.AP(tensor=sc_t.tensor,
                                        offset=T + mt * P * T,
                                        ap=[[T, P], [1, T]]))
                        rbf_tiles[(bh, mt)] = t

                def stage_b(bh, mt):
                    b, h = divmod(bh, H)
                    hsl = slice(32 * h, 32 * h + 32)
                    mg = slice(b * T + mt * P, b * T + (mt + 1) * P)
                    at = at_tiles[bh]
                    rec2 = rec_tiles[bh]
                    rbf = rbf_tiles.pop((bh, mt))
                    e8 = e8p.tile([P, T], FP8, tag="E8")
                    for nck in range(2):
                        cp = psc.tile([P, 512], F32, tag="c")
                        nc.tensor.matmul(
                            cp, lhsT=qu8[hsl, :, mg],
                            rhs=k8[hsl, :, b * T + nck * 512:b * T + (nck + 1) * 512],
                            start=True, stop=False, perf_mode=DR,
                            skip_group_check=True, tile_position=(32 * h, 0))
                        nc.tensor.matmul(
                            cp, lhsT=id_bf,
                            rhs=rbf[:, nck * 512:(nck + 1) * 512],
                            start=False, stop=True, skip_group_check=True)
                        nc.scalar.activation(
                            out=e8[:, nck * 512:(nck + 1) * 512], in_=cp,
                            func=EXP, scale=0.125,
                            accum_out=rec2[:, nck, mt:mt + 1])
                    nc.sync.dma_start_transpose(
                        out=at[:, :, mt * P:(mt + 1) * P], in_=e8.bitcast(U16))

                def stage_d_piece(bh, i, avp):
                    b, h = divmod(bh, H)
                    mc, c = i // 4, i % 4
                    at8 = at_tiles[bh].bitcast(FP8)
                    rhs = at8[:, c, mc * 1024:(mc + 1) * 1024].rearrange(
                        "p (m two) -> p two m", two=2)
                    nc.tensor.matmul(
                        avp[:, mc * 512:(mc + 1) * 512],
                        lhsT=v8p[:, b, c, :, 64 * h:64 * h + 64],
                        rhs=rhs, start=(c == 0), stop=(c == 3),
                        perf_mode=DR, skip_group_check=True)

                def stage_d2(bh, avp):
                    b, h = divmod(bh, H)
                    rec2 = rec_tiles.pop(bh)
                    recr = recp.tile([P, 8], F32, tag="recr")
                    nc.vector.tensor_tensor(out=recr, in0=rec2[:, 0, :],
                                            in1=rec2[:, 1, :], op=ADD)
                    nc.vector.reciprocal(out=recr, in_=recr)
                    rt = psx.tile([8, P], F32, tag="rt", bufs=1)
                    nc.tensor.transpose(rt, recr, ident)
                    recT = recp.tile([8, P], BF16, tag="recT")
                    nc.scalar.copy(out=recT, in_=rt)
                    recF = recp.tile([1, T], BF16, tag="recF")
                    nc.gpsimd.dma_start(out=recF, in_=recT)
                    recB = recp.tile([P, T], BF16, tag="recB")
                    nc.gpsimd.partition_broadcast(recB, recF)
                    for mc in range(2):
                        nc.vector.scalar_tensor_tensor(
                            out=avT[h][0:64,
                                       b * T + mc * 512:b * T + (mc + 1) * 512],
                            in0=avp[0:64, mc * 512:(mc + 1) * 512],
                            scalar=0.0,
                            in1=recB[0:64, mc * 512:(mc + 1) * 512],
                            op0=BYPASS, op1=MULT)
                    del at_tiles[bh]

                def ph4_body(mt):
                    op = ps4.tile([P, DM], F32, tag="op")
                    for h_ in range(4):
                        nc.tensor.matmul(op,
                                         lhsT=avT[h_][:, mt * P:(mt + 1) * P],
                                         rhs=wo_sb[h_],
                                         start=(h_ == 0), stop=(h_ == 3))
                    ot = sb3.tile([P, DM], F32, tag="ot")
                    nc.vector.scalar_tensor_tensor(out=ot, in0=op, scalar=0.0,
                                                   in1=x_res[:, mt, :],
                                                   op0=BYPASS, op1=ADD)
                    nc.vector.tensor_tensor(out=ot, in0=ot, in1=bo_b, op=ADD)
                    nc.sync.dma_start(out=out[mt * P:(mt + 1) * P, :], in_=ot)

                avp_cur = None
                for step in range(NBH + 2):
                    if 0 <= step - 1 < NBH:
                        at_tiles[step - 1] = atp.tile([P, 4, T], U16, tag="at",
                                                      name="at")
                        rec_tiles[step - 1] = recp.tile([P, 2, 8], F32,
                                                        tag="rec2",
                                                        name="rec2")
                    if step - 2 >= 0:
                        avp_cur = psav.tile([64, T], F32, tag="av", name="av")
                    for mt in range(8):
                        if step < NBH:
                            stage_a(step, mt)
                        if 0 <= step - 1 < NBH:
                            stage_b(step - 1, mt)
                        if step - 2 >= 0:
                            stage_d_piece(step - 2, mt, avp_cur)
                    if step < NBH:
                        issue_shears(step)
                    if step - 2 >= 0:
                        stage_d2(step - 2, avp_cur)
                    if step - 2 == 3:
                        for mt in range(8):
                            ph4_body(mt)
                for mt in range(8, 16):
                    ph4_body(mt)
    nc.finalize()
    return nc


_NC = None


def make_in_maps(inputs):
    f = lambda a: np.ascontiguousarray(np.asarray(a, dtype=np.float32))
    x = f(inputs["inputs"]).reshape(B, T, DM)
    pos = f(inputs["pos_enc"]).reshape(B, T, DM)
    wq0 = f(inputs["Wq"]).reshape(DM, DM)
    wk0 = f(inputs["Wk"]).reshape(DM, DM)
    wv0 = f(inputs["Wv"]).reshape(DM, DM)
    wp0 = f(inputs["Wp"]).reshape(DM, DM)
    wo0 = f(inputs["Wo"]).reshape(DM, DM)
    gamma = f(inputs["gamma"]).reshape(DM, 1)
    beta = f(inputs["beta"]).reshape(DM)

    # head-fold column permutation: pos j*128 + h*32 + k <- col h*64 + j*32 + k
    perm = np.array([h * 64 + j * 32 + k
                     for j in range(2) for h in range(H) for k in range(32)])

    # fold LN gamma into x-side weights, beta into biases, bv through the
    # (normalized) attention into the output bias
    wq, wk, wv = gamma * wq0, gamma * wk0, gamma * wv0
    bqu = (f(inputs["bq"]).reshape(DM) + f(inputs["pos_bias_u"]).reshape(DM)
           + beta @ wq0)
    bqv = (f(inputs["bq"]).reshape(DM) + f(inputs["pos_bias_v"]).reshape(DM)
           + beta @ wq0)
    bk = f(inputs["bk"]).reshape(DM) + beta @ wk0
    bv_eff = f(inputs["bv"]).reshape(DM) + beta @ wv0
    bo = f(inputs["bo"]) + bv_eff @ wo0

    c = np.ascontiguousarray
    wq_dr = c((wq[:, perm] * WSCALE).reshape(2, P, DM))
    wk_dr = c((wk[:, perm] * WSCALE).reshape(2, P, DM))
    wp_dr = c((wp0[:, perm] * WSCALE).reshape(2, P, DM))
    wv_dr = c((wv * WSCALE).reshape(2, P, DM))
    wo_dr = c(wo0.reshape(4, 64, DM))
    bqu_p, dqv_p, bk_p = bqu[perm], (bqv - bqu)[perm], bk[perm]
    bvec = c(np.stack([bqu_p[:P], bqu_p[P:], dqv_p[:P], dqv_p[P:],
                       bk_p[:P], bk_p[P:]]))

    shared = dict(wq=wq_dr, wk=wk_dr, wp=wp_dr, wv=wv_dr, wo=wo_dr,
                  bvec=bvec, bo=c(bo))
    in_maps = []
    for core in range(NCORES):
        sl = slice(core * BL, (core + 1) * BL)
        in_maps.append(dict(
            x=c(x[sl].reshape(M, DM)),
            pos=c(pos[sl].reshape(M, DM)),
            **shared,
        ))
    return in_maps


def kernel(**inputs) -> np.ndarray:
    global _NC
    if _NC is None:
        _NC = build_bass()
    in_maps = make_in_maps(inputs)
    res = run_bass_kernel_spmd(_NC, in_maps, core_ids=list(range(NCORES)))
    outs = [r["out"].reshape(BL, T, DM) for r in res.results]
    return np.concatenate(outs, axis=0)
